# revision 8
# baseline (speedup 1.0000x reference)
"""VQ codebook kernel (nn_CodeBook_12902081757285) for 8 Trainium2 NeuronCores.

Reference computation (forward only):
    z = lat.reshape(B, L//32, 32)               # [2048, 64, 32]
    d[r,k] = ||z_r||^2 - 2 z_r.c_k + ||c_k||^2  # over 1024 centroids
    cent_loss = (1 + BETA) * mean_over_rows(min_k d[r,k]) / 32
    out = lat * clip(leak_factor, 1e-3, 1e3)

Device strategy (data parallel over lat rows, 8 cores):
  - Each core gets 256 lat rows ([256, 2048] = 16384 z-rows).
  - PE transposes z blocks into zT form ([32, 128] stationaries, f32r),
    then one K=34 f32r matmul pair per 128 z-rows computes
    -2 z.c + ||c||^2 into PSUM [128, 1024] (ones rows x c2_hi/c2_lo fold
    the ||c||^2 term into the matmul; ||z||^2 is separable and summed
    exactly from the natural-layout fp32 tiles).
  - A custom fused DVE op (min(Src0,Src1) with a MIN accumulator) reduces
    each PSUM distance tile to a per-row minimum in a single pass.
  - Host sums the tiny per-core partials ([128,128] mins + [128,2] sumsq)
    and assembles the scalar loss; the scaled lat output is produced by
    DMA (lf == 1) or a scalar-engine multiply (lf != 1).
"""

import sys

sys.path.insert(0, "/opt/trn_rl_repo")

import numpy as np
from contextlib import ExitStack

import concourse.bacc as bacc
import concourse.tile as tile
from concourse import mybir
from concourse.bass_utils import run_bass_kernel_spmd
import concourse.dve_ops as dve_ops
from concourse.dve_ops import (
    DveOp,
    OPS,
    CUSTOM_DVE_SPECS,
    _SUB_OPCODE_FOR_NAME,
    _CUSTOM_DVE_ROW_BASE,
)
from concourse.dve_spec import Spec, Src0, Src1, C0, minn, lower, _has_src1
from concourse.dve_uop import DveOpSpec

F32R = mybir.dt.float32r
F32 = mybir.dt.float32

NCORES = 8
B, L, D, K = 2048, 2048, 32, 1024
BETA = 0.25
RPC = B // NCORES            # lat rows per core (256)
NATT = RPC // 128            # natural [128, 2048] tiles per core (2)
NT = RPC * (L // D) // 128   # 128-z-row tiles per core (128)
ZT_RING = 6                  # fixed zt stationary slots (ones rows persist)


def _register_op(name, spec):
    if name in _SUB_OPCODE_FOR_NAME:
        return next(o for o in OPS if o.name == name)
    row = _CUSTOM_DVE_ROW_BASE + len(OPS)
    shas = {
        ver: DveOpSpec(
            name=name, opcode=row, uops=lower(spec, ver=ver), rd1_en=_has_src1(spec)
        ).sha(ver)
        for ver in ("v3", "v4")
    }
    op = DveOp(name, spec, subdim=False, uops_sha=shas)
    OPS.append(op)
    CUSTOM_DVE_SPECS[name] = spec
    _SUB_OPCODE_FOR_NAME[name] = row
    return op


def _register_min_ops():
    """Fused min-reduce DVE ops.

    MIN1: out = in0;           accum_out = min(s0, min_row(in0))
    MIN2: out = min(in0, in1); accum_out = min(s0, min_row(out))
    """

    def _ref1(in0, c0, c1, c2):
        b = in0.astype(np.float32)
        return b, np.minimum(c0, b.reshape(b.shape[0], -1).min(axis=-1, keepdims=True))

    def _ref2(in0, in1, c0, c1, c2):
        b = np.minimum(in0.astype(np.float32), in1.astype(np.float32))
        return b, np.minimum(c0, b.reshape(b.shape[0], -1).min(axis=-1, keepdims=True))

    min1 = _register_op(
        "ANT_MIN1_REDUCE_VQ",
        Spec(body=minn(Src0, C0), accum=minn, accum_init=C0, reference=None),
    )
    min2 = _register_op(
        "ANT_MIN2_REDUCE_VQ",
        Spec(body=minn(Src0, Src1), accum=minn, accum_init=C0, reference=_ref2),
    )
    return min1, min2


# Fraction of distance tiles min-reduced directly from PSUM by the DVE; the
# rest are converted fp32->bf16 by the scalar engine and min-reduced from SBUF
# (two streams) by the DVE. Balances DVE vs ACT occupancy.
DIRECT_NUM, DIRECT_DEN = 5, 12


def _build(lf: float):
    MIN1, MIN2 = _register_min_ops()
    nc = bacc.Bacc("TRN2", target_bir_lowering=False, debug=False)

    lat_in = nc.dram_tensor("lat_in", [RPC, L], F32, kind="ExternalInput").ap()
    ct_in = nc.dram_tensor("ct_in", [34, K], F32R, kind="ExternalInput").ap()
    id_in = nc.dram_tensor("id_in", [128, 128], F32R, kind="ExternalInput").ap()
    ones_in = nc.dram_tensor("ones_in", [2, 512], F32R, kind="ExternalInput").ap()
    out_lat = nc.dram_tensor("out_lat", [RPC, L], F32, kind="ExternalOutput").ap()
    m_out = nc.dram_tensor("m_out", [128, NT], F32, kind="ExternalOutput").ap()
    z2_out = nc.dram_tensor("z2_out", [128, NATT], F32, kind="ExternalOutput").ap()

    with tile.TileContext(nc) as tc, ExitStack() as ctx:
        singles = ctx.enter_context(tc.tile_pool(name="singles", bufs=1))
        nat_pool = ctx.enter_context(tc.tile_pool(name="nat", bufs=2))
        sc_pool = ctx.enter_context(tc.tile_pool(name="scaled", bufs=2))
        ps_t = ctx.enter_context(tc.tile_pool(name="pst", bufs=2, space="PSUM"))
        ps_d = ctx.enter_context(tc.tile_pool(name="psd", bufs=2, space="PSUM"))

        ct = singles.tile([34, K], F32R, tag="ct")
        ident = singles.tile([128, 128], F32R, tag="ident")
        mbuf = singles.tile([128, NT], F32, tag="mbuf")
        z2buf = singles.tile([128, NATT], F32, tag="z2buf")
        mscr = singles.tile([128, K], F32, tag="mscr")
        mscr2 = singles.tile([128, 512], mybir.dt.bfloat16, tag="mscr2")
        z2scr = singles.tile([128, L], F32, tag="z2scr")
        cvt_pool = ctx.enter_context(tc.tile_pool(name="cvt", bufs=3))
        nc.sync.dma_start(ct[:], ct_in[:])
        nc.sync.dma_start(ident[:], id_in[:])

        zt_slots = []
        for s in range(ZT_RING):
            zt = singles.tile([34, 512], F32R, tag=f"zt{s}")
            nc.sync.dma_start(zt[32:34, :], ones_in[:])
            zt_slots.append(zt)

        # main output: exact passthrough (lf == 1) straight DRAM->DRAM
        if lf == 1.0:
            for i in range(NATT):
                nc.sync.dma_start(
                    out_lat[i * 128 : (i + 1) * 128, :],
                    lat_in[i * 128 : (i + 1) * 128, :],
                )

        for i in range(NATT):
            # f32r view of lat for the PE (DMA rounds fp32 -> f32r in flight);
            # the loss is then computed exactly on the rounded z', which only
            # perturbs the loss at the ~2^-12 relative level.
            nat = nat_pool.tile([128, L], F32R)
            nc.gpsimd.dma_start(nat[:], lat_in[i * 128 : (i + 1) * 128, :])
            natf = nat[:].bitcast(F32)

            # sum(z'^2) per partition, one column per natural tile
            nc.vector._custom_dve(
                dve_ops.TENSOR_TENSOR_REDUCE,
                out=z2scr[:],
                in0=natf,
                in1=natf,
                s0=0.0,
                s1=1.0,
                accum_out=z2buf[:, i : i + 1],
            )

            if lf != 1.0:
                # exact scaled output needs unrounded lat
                natx = sc_pool.tile([128, L], F32)
                nc.sync.dma_start(natx[:], lat_in[i * 128 : (i + 1) * 128, :])
                onat = sc_pool.tile([128, L], F32)
                nc.scalar.mul(onat[:], natx[:], lf)
                nc.sync.dma_start(out_lat[i * 128 : (i + 1) * 128, :], onat[:])

            for g in range(L // 128):  # 16 column blocks of 128 (= 4 z-groups)
                pst = ps_t.tile([32, 512], F32R)
                for q in range(4):
                    nc.tensor.transpose(
                        pst[:, q * 128 : (q + 1) * 128],
                        nat[:, (g * 4 + q) * 32 : (g * 4 + q + 1) * 32],
                        ident[:],
                    )
                zt = zt_slots[(i * (L // 128) + g) % ZT_RING]
                nc.scalar.copy(zt[0:32, :], pst[:])

                for q in range(4):
                    t = (i * (L // 128) + g) * 4 + q
                    pd = ps_d.tile([128, K], F32)
                    lhsT = zt[:, q * 128 : (q + 1) * 128]
                    nc.tensor.matmul(pd[:, 0:512], lhsT, ct[:, 0:512], start=True, stop=True)
                    nc.tensor.matmul(pd[:, 512:1024], lhsT, ct[:, 512:1024], start=True, stop=True)
                    if (t * DIRECT_NUM) % DIRECT_DEN < DIRECT_NUM:
                        # DVE min-reduces the whole tile straight from PSUM
                        nc.vector._custom_dve(
                            MIN1,
                            out=mscr[:],
                            in0=pd[:],
                            s0=3.0e38,
                            accum_out=mbuf[:, t : t + 1],
                        )
                    else:
                        # ACT converts to bf16 in SBUF; DVE reduces two streams
                        cvt = cvt_pool.tile([128, K], mybir.dt.bfloat16)
                        nc.scalar.copy(cvt[:], pd[:])
                        nc.vector._custom_dve(
                            MIN2,
                            out=mscr2[:],
                            in0=cvt[:, 0:512],
                            in1=cvt[:, 512:1024],
                            s0=3.0e38,
                            accum_out=mbuf[:, t : t + 1],
                        )

        nc.sync.dma_start(m_out[:], mbuf[:])
        nc.sync.dma_start(z2_out[:], z2buf[:])

    nc.compile()
    return nc


_NC_CACHE = {}


def kernel(lat, centroids, leak_factor, _want_trace=False):
    lat = np.ascontiguousarray(np.asarray(lat, dtype=np.float32))
    centroids = np.ascontiguousarray(np.asarray(centroids, dtype=np.float32))
    lf = float(np.clip(np.float32(np.asarray(leak_factor)), 1e-3, 1e3))

    if lf not in _NC_CACHE:
        _NC_CACHE[lf] = _build(lf)
    nc = _NC_CACHE[lf]

    # moving operand: rows 0..31 = -2*c^T, rows 32/33 = ||c||^2 split hi/lo
    c2 = (centroids.astype(np.float64) ** 2).sum(axis=1)
    c2_hi = c2.astype(np.float32)
    c2_lo = (c2 - c2_hi).astype(np.float32)
    ct = np.concatenate(
        [(-2.0 * centroids.T).astype(np.float32), c2_hi[None, :], c2_lo[None, :]], axis=0
    )  # [34, 1024]
    ident = np.eye(128, dtype=np.float32)
    ones = np.ones((2, 512), dtype=np.float32)

    in_maps = [
        {
            "lat_in": lat[c * RPC : (c + 1) * RPC],
            "ct_in": ct,
            "id_in": ident,
            "ones_in": ones,
        }
        for c in range(NCORES)
    ]
    res = run_bass_kernel_spmd(
        nc, in_maps, core_ids=list(range(NCORES)), trace=_want_trace
    )

    out = np.concatenate([r["out_lat"] for r in res.results], axis=0)
    msum = np.float64(0.0)
    z2sum = np.float64(0.0)
    for r in res.results:
        msum += r["m_out"].astype(np.float64).sum()
        z2sum += r["z2_out"].astype(np.float64).sum()
    loss = np.float32((1.0 + BETA) * (z2sum + msum) / (B * L))
    kernel.last_results = res
    return out, loss


# revision 32
# speedup vs baseline: 1.1191x; 1.1191x over previous
"""VQ codebook kernel (nn_CodeBook_12902081757285) for 8 Trainium2 NeuronCores.

Reference computation (forward only):
    z = lat.reshape(B, L//32, 32)               # [2048, 64, 32]
    d[r,k] = ||z_r||^2 - 2 z_r.c_k + ||c_k||^2  # over 1024 centroids
    cent_loss = (1 + BETA) * mean_over_rows(min_k d[r,k]) / 32
    out = lat * clip(leak_factor, 1e-3, 1e3)

Device strategy (data parallel over lat rows, 8 cores):
  - Each core gets 256 lat rows ([256, 2048] = 16384 z-rows).
  - PE transposes z blocks into zT form ([32, 128] stationaries, f32r),
    then one K=34 f32r matmul pair per 128 z-rows computes
    -2 z.c + ||c||^2 into PSUM [128, 1024] (ones rows x c2_hi/c2_lo fold
    the ||c||^2 term into the matmul; ||z||^2 is separable and summed
    exactly from the natural-layout fp32 tiles).
  - A custom fused DVE op (min(Src0,Src1) with a MIN accumulator) reduces
    each PSUM distance tile to a per-row minimum in a single pass.
  - Host sums the tiny per-core partials ([128,128] mins + [128,2] sumsq)
    and assembles the scalar loss; the scaled lat output is produced by
    DMA (lf == 1) or a scalar-engine multiply (lf != 1).
"""

import sys

sys.path.insert(0, "/opt/trn_rl_repo")

import numpy as np
from contextlib import ExitStack

import concourse.bacc as bacc
import concourse.tile as tile
from concourse import mybir
from concourse.bass_utils import run_bass_kernel_spmd
import concourse.dve_ops as dve_ops
from concourse.dve_ops import (
    DveOp,
    OPS,
    CUSTOM_DVE_SPECS,
    _SUB_OPCODE_FOR_NAME,
    _CUSTOM_DVE_ROW_BASE,
)
from concourse.dve_spec import Spec, Src0, Src1, C0, minn, lower, _has_src1
from concourse.dve_uop import DveOpSpec

F32R = mybir.dt.float32r
F32 = mybir.dt.float32

NCORES = 8
B, L, D, K = 2048, 2048, 32, 1024
BETA = 0.25
RPC = B // NCORES            # lat rows per core (256)
NATT = RPC // 128            # natural [128, 2048] tiles per core (2)
NT = RPC * (L // D) // 128   # 128-z-row tiles per core (128)
ZT_RING = 10                 # fixed zt stationary slots (ones rows persist)


def _register_op(name, spec):
    if name in _SUB_OPCODE_FOR_NAME:
        return next(o for o in OPS if o.name == name)
    row = _CUSTOM_DVE_ROW_BASE + len(OPS)
    shas = {
        ver: DveOpSpec(
            name=name, opcode=row, uops=lower(spec, ver=ver), rd1_en=_has_src1(spec)
        ).sha(ver)
        for ver in ("v3", "v4")
    }
    op = DveOp(name, spec, subdim=False, uops_sha=shas)
    OPS.append(op)
    CUSTOM_DVE_SPECS[name] = spec
    _SUB_OPCODE_FOR_NAME[name] = row
    return op


def _register_min_ops():
    """Fused min-reduce DVE ops.

    MIN1: out = in0;           accum_out = min(s0, min_row(in0))
    MIN2: out = min(in0, in1); accum_out = min(s0, min_row(out))
    """

    def _ref1(in0, c0, c1, c2):
        b = in0.astype(np.float32)
        return b, np.minimum(c0, b.reshape(b.shape[0], -1).min(axis=-1, keepdims=True))

    def _ref2(in0, in1, c0, c1, c2):
        b = np.minimum(in0.astype(np.float32), in1.astype(np.float32))
        return b, np.minimum(c0, b.reshape(b.shape[0], -1).min(axis=-1, keepdims=True))

    min1 = _register_op(
        "ANT_MIN1_REDUCE_VQ",
        Spec(body=minn(Src0, C0), accum=minn, accum_init=C0, reference=None),
    )
    min2 = _register_op(
        "ANT_MIN2_REDUCE_VQ",
        Spec(body=minn(Src0, Src1), accum=minn, accum_init=C0, reference=_ref2),
    )
    return min1, min2


# Per-tile reduction strategy schedule, balancing DVE / ACT / GPSIMD load:
#   'a': DVE min-reduces the whole [128,1024] tile straight from PSUM
#   'b': ACT converts the right half to bf16 SBUF; DVE reduces PSUM+SBUF
#   'c': ACT converts the whole tile to bf16; GPSIMD folds 1024->256;
#        DVE finishes 256->per-row-min
PATTERN = "b"  # all-b balances DVE (84us) vs ACT (99us) best per TimelineSim
Z2_ON_GPSIMD = False  # walrus rejects TensorScalarPtr on Pool; DVE has headroom
CVT_BUFS = 6
GP_BUFS = 3


def _build(lf: float, pattern=None, z2_gp=None, cvt_bufs=None, gp_bufs=None, zt_copy='act', pst_bufs=2, psd_bufs=3, zt_ring=None, use_psa=False):
    pattern = PATTERN if pattern is None else pattern
    zt_ring = ZT_RING if zt_ring is None else zt_ring
    z2_gp = Z2_ON_GPSIMD if z2_gp is None else z2_gp
    cvt_bufs = CVT_BUFS if cvt_bufs is None else cvt_bufs
    gp_bufs = GP_BUFS if gp_bufs is None else gp_bufs
    MIN1, MIN2 = _register_min_ops()
    nc = bacc.Bacc("TRN2", target_bir_lowering=False, debug=False)

    lat_in = nc.dram_tensor("lat_in", [RPC, L], F32, kind="ExternalInput").ap()
    ct_in = nc.dram_tensor("ct_in", [34, K], F32R, kind="ExternalInput").ap()
    id_in = nc.dram_tensor("id_in", [128, 128], F32R, kind="ExternalInput").ap()
    ones_in = nc.dram_tensor("ones_in", [2, 512], F32R, kind="ExternalInput").ap()
    out_lat = nc.dram_tensor("out_lat", [RPC, L], F32, kind="ExternalOutput").ap()
    m_out = nc.dram_tensor("m_out", [128, NT], F32, kind="ExternalOutput").ap()
    z2_out = nc.dram_tensor("z2_out", [128, NATT], F32, kind="ExternalOutput").ap()

    with tile.TileContext(nc) as tc, ExitStack() as ctx:
        singles = ctx.enter_context(tc.tile_pool(name="singles", bufs=1))
        nat_pool = ctx.enter_context(tc.tile_pool(name="nat", bufs=2))
        sc_pool = ctx.enter_context(tc.tile_pool(name="scaled", bufs=2))
        ps_t = ctx.enter_context(tc.tile_pool(name="pst", bufs=pst_bufs, space="PSUM"))
        ps_d = ctx.enter_context(tc.tile_pool(name="psd", bufs=psd_bufs, space="PSUM"))
        ps_a = (
            ctx.enter_context(tc.tile_pool(name="psa", bufs=1, space="PSUM"))
            if use_psa
            else ps_d
        )

        ct = singles.tile([34, K], F32R, tag="ct")
        ident = singles.tile([128, 128], F32R, tag="ident")
        mbuf = singles.tile([128, NT], F32, tag="mbuf")
        z2buf = singles.tile([128, NATT], F32, tag="z2buf")
        mscr = singles.tile([128, K], F32, tag="mscr")
        mscr2 = singles.tile([128, 512], mybir.dt.bfloat16, tag="mscr2")
        mscr3 = singles.tile([128, 128], mybir.dt.bfloat16, tag="mscr3")
        z2scr = singles.tile([128, L], F32, tag="z2scr")
        cvt_pool = ctx.enter_context(tc.tile_pool(name="cvt", bufs=cvt_bufs))
        gp_pool = ctx.enter_context(tc.tile_pool(name="gp", bufs=gp_bufs))
        nc.sync.dma_start(ct[:], ct_in[:])
        nc.sync.dma_start(ident[:], id_in[:])

        zt_slots = []
        for s in range(zt_ring):
            zt = singles.tile([34, 512], F32R, tag=f"zt{s}")
            nc.sync.dma_start(zt[32:34, :], ones_in[:])
            zt_slots.append(zt)

        # main output: exact passthrough (lf == 1) straight DRAM->DRAM
        if lf == 1.0:
            for i in range(NATT):
                nc.sync.dma_start(
                    out_lat[i * 128 : (i + 1) * 128, :],
                    lat_in[i * 128 : (i + 1) * 128, :],
                )

        for i in range(NATT):
            # f32r view of lat for the PE (DMA rounds fp32 -> f32r in flight);
            # the loss is then computed exactly on the rounded z', which only
            # perturbs the loss at the ~2^-12 relative level.
            nat = nat_pool.tile([128, L], F32R)
            for ch in range(4):  # chunked so transposes start early
                nc.gpsimd.dma_start(
                    nat[:, ch * 512 : (ch + 1) * 512],
                    lat_in[i * 128 : (i + 1) * 128, ch * 512 : (ch + 1) * 512],
                )
            natf = nat[:].bitcast(F32)

            # sum(z'^2) per partition, one column per natural tile
            if z2_gp:
                nc.gpsimd.scalar_tensor_tensor(
                    out=z2scr[:],
                    in0=natf,
                    scalar=0.0,
                    in1=natf,
                    op0=mybir.AluOpType.add,
                    op1=mybir.AluOpType.mult,
                    accum_out=z2buf[:, i : i + 1],
                )
            else:
                nc.vector._custom_dve(
                    dve_ops.TENSOR_TENSOR_REDUCE,
                    out=z2scr[:],
                    in0=natf,
                    in1=natf,
                    s0=0.0,
                    s1=1.0,
                    accum_out=z2buf[:, i : i + 1],
                )

            if lf != 1.0:
                # exact scaled output needs unrounded lat
                natx = sc_pool.tile([128, L], F32)
                nc.sync.dma_start(natx[:], lat_in[i * 128 : (i + 1) * 128, :])
                onat = sc_pool.tile([128, L], F32)
                nc.scalar.mul(onat[:], natx[:], lf)
                nc.sync.dma_start(out_lat[i * 128 : (i + 1) * 128, :], onat[:])

            def emit_stage(g):
                """Transpose 4 z-groups of block g and copy into a zt slot."""
                pst = ps_t.tile([32, 512], F32R)
                for q in range(4):
                    nc.tensor.transpose(
                        pst[:, q * 128 : (q + 1) * 128],
                        nat[:, (g * 4 + q) * 32 : (g * 4 + q + 1) * 32],
                        ident[:],
                    )
                zt = zt_slots[(i * (L // 128) + g) % zt_ring]
                if zt_copy == "act" or (zt_copy == "alt" and g % 2 == 0):
                    nc.scalar.copy(zt[0:32, :], pst[:])
                else:
                    nc.vector.tensor_copy(zt[0:32, :], pst[:])
                return zt

            nblk = L // 128  # 16 column blocks of 128 (= 4 z-row-tiles each)
            zt_next = emit_stage(0)
            for g in range(nblk):
                zt = zt_next
                if g + 1 < nblk:
                    zt_next = emit_stage(g + 1)

                for q in range(4):
                    t = (i * (L // 128) + g) * 4 + q
                    cls = pattern[t % len(pattern)]
                    pd = (ps_a if cls == "a" else ps_d).tile([128, K], F32)
                    lhsT = zt[:, q * 128 : (q + 1) * 128]
                    nc.tensor.matmul(pd[:, 0:512], lhsT, ct[:, 0:512], start=True, stop=True)
                    nc.tensor.matmul(pd[:, 512:1024], lhsT, ct[:, 512:1024], start=True, stop=True)
                    if cls == "a":
                        # DVE min-reduces the whole tile straight from PSUM
                        nc.vector._custom_dve(
                            MIN1,
                            out=mscr[:],
                            in0=pd[:],
                            s0=3.0e38,
                            accum_out=mbuf[:, t : t + 1],
                        )
                    elif cls == "b":
                        # ACT converts right half; DVE reduces PSUM + SBUF
                        cvtb = cvt_pool.tile([128, 512], mybir.dt.bfloat16, tag="cvtb")
                        nc.scalar.copy(cvtb[:], pd[:, 512:1024])
                        nc.vector._custom_dve(
                            MIN2,
                            out=mscr2[:],
                            in0=pd[:, 0:512],
                            in1=cvtb[:],
                            s0=3.0e38,
                            accum_out=mbuf[:, t : t + 1],
                        )
                    else:
                        # ACT converts whole tile; GPSIMD folds 1024 -> 256;
                        # DVE finishes with the fused 2-stream min-reduce
                        cvt = cvt_pool.tile([128, K], mybir.dt.bfloat16, tag="cvtc")
                        nc.scalar.copy(cvt[:], pd[:])
                        gt1 = gp_pool.tile([128, 512], mybir.dt.bfloat16, tag="g1")
                        nc.gpsimd.tensor_tensor(
                            gt1[:], cvt[:, 0:512], cvt[:, 512:1024], mybir.AluOpType.min
                        )
                        gt2 = gp_pool.tile([128, 256], mybir.dt.bfloat16, tag="g2")
                        nc.gpsimd.tensor_tensor(
                            gt2[:], gt1[:, 0:256], gt1[:, 256:512], mybir.AluOpType.min
                        )
                        nc.vector._custom_dve(
                            MIN2,
                            out=mscr3[:],
                            in0=gt2[:, 0:128],
                            in1=gt2[:, 128:256],
                            s0=3.0e38,
                            accum_out=mbuf[:, t : t + 1],
                        )

        nc.sync.dma_start(m_out[:], mbuf[:])
        nc.sync.dma_start(z2_out[:], z2buf[:])

    nc.compile()
    return nc


_NC_CACHE = {}


def kernel(lat, centroids, leak_factor, _want_trace=False):
    lat = np.ascontiguousarray(np.asarray(lat, dtype=np.float32))
    centroids = np.ascontiguousarray(np.asarray(centroids, dtype=np.float32))
    lf = float(np.clip(np.float32(np.asarray(leak_factor)), 1e-3, 1e3))

    if lf not in _NC_CACHE:
        _NC_CACHE[lf] = _build(lf)
    nc = _NC_CACHE[lf]

    # moving operand: rows 0..31 = -2*c^T, rows 32/33 = ||c||^2 split hi/lo
    c2 = (centroids.astype(np.float64) ** 2).sum(axis=1)
    c2_hi = c2.astype(np.float32)
    c2_lo = (c2 - c2_hi).astype(np.float32)
    ct = np.concatenate(
        [(-2.0 * centroids.T).astype(np.float32), c2_hi[None, :], c2_lo[None, :]], axis=0
    )  # [34, 1024]
    ident = np.eye(128, dtype=np.float32)
    ones = np.ones((2, 512), dtype=np.float32)

    in_maps = [
        {
            "lat_in": lat[c * RPC : (c + 1) * RPC],
            "ct_in": ct,
            "id_in": ident,
            "ones_in": ones,
        }
        for c in range(NCORES)
    ]
    res = run_bass_kernel_spmd(
        nc, in_maps, core_ids=list(range(NCORES)), trace=_want_trace
    )

    out = np.concatenate([r["out_lat"] for r in res.results], axis=0)
    msum = np.float64(0.0)
    z2sum = np.float64(0.0)
    for r in res.results:
        msum += r["m_out"].astype(np.float64).sum()
        z2sum += r["z2_out"].astype(np.float64).sum()
    loss = np.float32((1.0 + BETA) * (z2sum + msum) / (B * L))
    kernel.last_results = res
    return out, loss


# revision 33
# speedup vs baseline: 1.1391x; 1.0179x over previous
"""VQ codebook kernel (nn_CodeBook_12902081757285) for 8 Trainium2 NeuronCores.

Reference computation (forward only):
    z = lat.reshape(B, L//32, 32)               # [2048, 64, 32]
    d[r,k] = ||z_r||^2 - 2 z_r.c_k + ||c_k||^2  # over 1024 centroids
    cent_loss = (1 + BETA) * mean_over_rows(min_k d[r,k]) / 32
    out = lat * clip(leak_factor, 1e-3, 1e3)

Device strategy (data parallel over lat rows, 8 cores):
  - Each core gets 256 lat rows ([256, 2048] = 16384 z-rows).
  - PE transposes z blocks into zT form ([32, 128] stationaries, f32r),
    then one K=34 f32r matmul pair per 128 z-rows computes
    -2 z.c + ||c||^2 into PSUM [128, 1024] (ones rows x c2_hi/c2_lo fold
    the ||c||^2 term into the matmul; ||z||^2 is separable and summed
    exactly from the natural-layout fp32 tiles).
  - A custom fused DVE op (min(Src0,Src1) with a MIN accumulator) reduces
    each PSUM distance tile to a per-row minimum in a single pass.
  - Host sums the tiny per-core partials ([128,128] mins + [128,2] sumsq)
    and assembles the scalar loss; the scaled lat output is produced by
    DMA (lf == 1) or a scalar-engine multiply (lf != 1).
"""

import sys

sys.path.insert(0, "/opt/trn_rl_repo")

import numpy as np
from contextlib import ExitStack

import concourse.bacc as bacc
import concourse.tile as tile
from concourse import mybir
from concourse.bass_utils import run_bass_kernel_spmd
import concourse.dve_ops as dve_ops
from concourse.dve_ops import (
    DveOp,
    OPS,
    CUSTOM_DVE_SPECS,
    _SUB_OPCODE_FOR_NAME,
    _CUSTOM_DVE_ROW_BASE,
)
from concourse.dve_spec import Spec, Src0, Src1, C0, minn, lower, _has_src1
from concourse.dve_uop import DveOpSpec

F32R = mybir.dt.float32r
F32 = mybir.dt.float32

NCORES = 8
B, L, D, K = 2048, 2048, 32, 1024
BETA = 0.25
RPC = B // NCORES            # lat rows per core (256)
NATT = RPC // 128            # natural [128, 2048] tiles per core (2)
NT = RPC * (L // D) // 128   # 128-z-row tiles per core (128)
ZT_RING = 10                 # fixed zt stationary slots (ones rows persist)


def _register_op(name, spec):
    if name in _SUB_OPCODE_FOR_NAME:
        return next(o for o in OPS if o.name == name)
    row = _CUSTOM_DVE_ROW_BASE + len(OPS)
    shas = {
        ver: DveOpSpec(
            name=name, opcode=row, uops=lower(spec, ver=ver), rd1_en=_has_src1(spec)
        ).sha(ver)
        for ver in ("v3", "v4")
    }
    op = DveOp(name, spec, subdim=False, uops_sha=shas)
    OPS.append(op)
    CUSTOM_DVE_SPECS[name] = spec
    _SUB_OPCODE_FOR_NAME[name] = row
    return op


def _register_min_ops():
    """Fused min-reduce DVE ops.

    MIN1: out = in0;           accum_out = min(s0, min_row(in0))
    MIN2: out = min(in0, in1); accum_out = min(s0, min_row(out))
    """

    def _ref1(in0, c0, c1, c2):
        b = in0.astype(np.float32)
        return b, np.minimum(c0, b.reshape(b.shape[0], -1).min(axis=-1, keepdims=True))

    def _ref2(in0, in1, c0, c1, c2):
        b = np.minimum(in0.astype(np.float32), in1.astype(np.float32))
        return b, np.minimum(c0, b.reshape(b.shape[0], -1).min(axis=-1, keepdims=True))

    min1 = _register_op(
        "ANT_MIN1_REDUCE_VQ",
        Spec(body=minn(Src0, C0), accum=minn, accum_init=C0, reference=None),
    )
    min2 = _register_op(
        "ANT_MIN2_REDUCE_VQ",
        Spec(body=minn(Src0, Src1), accum=minn, accum_init=C0, reference=_ref2),
    )
    return min1, min2


# Per-tile reduction strategy schedule, balancing DVE / ACT / GPSIMD load:
#   'a': DVE min-reduces the whole [128,1024] tile straight from PSUM
#   'b': ACT converts the right half to bf16 SBUF; DVE reduces PSUM+SBUF
#   'c': ACT converts the whole tile to bf16; GPSIMD folds 1024->256;
#        DVE finishes 256->per-row-min
PATTERN = "b"  # all-b balances DVE (84us) vs ACT (99us) best per TimelineSim
Z2_ON_GPSIMD = False  # walrus rejects TensorScalarPtr on Pool; DVE has headroom
CVT_BUFS = 6
GP_BUFS = 3


def _build(lf: float, pattern=None, z2_gp=None, cvt_bufs=None, gp_bufs=None, zt_copy='act', pst_bufs=2, psd_bufs=3, zt_ring=None, use_psa=False):
    pattern = PATTERN if pattern is None else pattern
    zt_ring = ZT_RING if zt_ring is None else zt_ring
    z2_gp = Z2_ON_GPSIMD if z2_gp is None else z2_gp
    cvt_bufs = CVT_BUFS if cvt_bufs is None else cvt_bufs
    gp_bufs = GP_BUFS if gp_bufs is None else gp_bufs
    MIN1, MIN2 = _register_min_ops()
    nc = bacc.Bacc("TRN2", target_bir_lowering=False, debug=False)

    lat_in = nc.dram_tensor("lat_in", [RPC, L], F32, kind="ExternalInput").ap()
    ct_in = nc.dram_tensor("ct_in", [34, K], F32R, kind="ExternalInput").ap()
    id_in = nc.dram_tensor("id_in", [128, 128], F32R, kind="ExternalInput").ap()
    ones_in = nc.dram_tensor("ones_in", [2, 512], F32R, kind="ExternalInput").ap()
    out_lat = nc.dram_tensor("out_lat", [RPC, L], F32, kind="ExternalOutput").ap()
    m_out = nc.dram_tensor("m_out", [128, NT], F32, kind="ExternalOutput").ap()
    z2_out = nc.dram_tensor("z2_out", [128, NATT * 4], F32, kind="ExternalOutput").ap()

    with tile.TileContext(nc) as tc, ExitStack() as ctx:
        singles = ctx.enter_context(tc.tile_pool(name="singles", bufs=1))
        nat_pool = ctx.enter_context(tc.tile_pool(name="nat", bufs=2))
        sc_pool = ctx.enter_context(tc.tile_pool(name="scaled", bufs=2))
        ps_t = ctx.enter_context(tc.tile_pool(name="pst", bufs=pst_bufs, space="PSUM"))
        ps_d = ctx.enter_context(tc.tile_pool(name="psd", bufs=psd_bufs, space="PSUM"))
        ps_a = (
            ctx.enter_context(tc.tile_pool(name="psa", bufs=1, space="PSUM"))
            if use_psa
            else ps_d
        )

        ct = singles.tile([34, K], F32R, tag="ct")
        ident = singles.tile([128, 128], F32R, tag="ident")
        mbuf = singles.tile([128, NT], F32, tag="mbuf")
        z2buf = singles.tile([128, NATT * 4], F32, tag="z2buf")
        mscr = singles.tile([128, K], F32, tag="mscr")
        mscr2 = singles.tile([128, 512], mybir.dt.bfloat16, tag="mscr2")
        mscr3 = singles.tile([128, 128], mybir.dt.bfloat16, tag="mscr3")
        z2scr = singles.tile([128, 512], F32, tag="z2scr")
        cvt_pool = ctx.enter_context(tc.tile_pool(name="cvt", bufs=cvt_bufs))
        gp_pool = ctx.enter_context(tc.tile_pool(name="gp", bufs=gp_bufs))
        nc.sync.dma_start(ct[:], ct_in[:])
        nc.sync.dma_start(ident[:], id_in[:])

        zt_slots = []
        for s in range(zt_ring):
            zt = singles.tile([34, 512], F32R, tag=f"zt{s}")
            nc.sync.dma_start(zt[32:34, :], ones_in[:])
            zt_slots.append(zt)

        # main output: exact passthrough (lf == 1) straight DRAM->DRAM
        if lf == 1.0:
            for i in range(NATT):
                nc.sync.dma_start(
                    out_lat[i * 128 : (i + 1) * 128, :],
                    lat_in[i * 128 : (i + 1) * 128, :],
                )

        for i in range(NATT):
            # f32r view of lat for the PE (DMA rounds fp32 -> f32r in flight);
            # the loss is then computed exactly on the rounded z', which only
            # perturbs the loss at the ~2^-12 relative level.
            nat = nat_pool.tile([128, L], F32R)
            for ch in range(4):  # chunked so transposes start early
                nc.gpsimd.dma_start(
                    nat[:, ch * 512 : (ch + 1) * 512],
                    lat_in[i * 128 : (i + 1) * 128, ch * 512 : (ch + 1) * 512],
                )
            natf = nat[:].bitcast(F32)

            # sum(z'^2) per partition; one column per 512-wide chunk so the
            # ops interleave with the min-reduce stream instead of one long op
            for ch in range(4):
                nf = natf[:, ch * 512 : (ch + 1) * 512]
                nc.vector._custom_dve(
                    dve_ops.TENSOR_TENSOR_REDUCE,
                    out=z2scr[:],
                    in0=nf,
                    in1=nf,
                    s0=0.0,
                    s1=1.0,
                    accum_out=z2buf[:, i * 4 + ch : i * 4 + ch + 1],
                )

            if lf != 1.0:
                # exact scaled output needs unrounded lat
                natx = sc_pool.tile([128, L], F32)
                nc.sync.dma_start(natx[:], lat_in[i * 128 : (i + 1) * 128, :])
                onat = sc_pool.tile([128, L], F32)
                nc.scalar.mul(onat[:], natx[:], lf)
                nc.sync.dma_start(out_lat[i * 128 : (i + 1) * 128, :], onat[:])

            def emit_stage(g):
                """Transpose 4 z-groups of block g and copy into a zt slot."""
                pst = ps_t.tile([32, 512], F32R)
                for q in range(4):
                    nc.tensor.transpose(
                        pst[:, q * 128 : (q + 1) * 128],
                        nat[:, (g * 4 + q) * 32 : (g * 4 + q + 1) * 32],
                        ident[:],
                    )
                zt = zt_slots[(i * (L // 128) + g) % zt_ring]
                if zt_copy == "act" or (zt_copy == "alt" and g % 2 == 0):
                    nc.scalar.copy(zt[0:32, :], pst[:])
                else:
                    nc.vector.tensor_copy(zt[0:32, :], pst[:])
                return zt

            nblk = L // 128  # 16 column blocks of 128 (= 4 z-row-tiles each)
            zt_next = emit_stage(0)
            for g in range(nblk):
                zt = zt_next
                if g + 1 < nblk:
                    zt_next = emit_stage(g + 1)

                for q in range(4):
                    t = (i * (L // 128) + g) * 4 + q
                    cls = pattern[t % len(pattern)]
                    pd = (ps_a if cls == "a" else ps_d).tile([128, K], F32)
                    lhsT = zt[:, q * 128 : (q + 1) * 128]
                    nc.tensor.matmul(pd[:, 0:512], lhsT, ct[:, 0:512], start=True, stop=True)
                    nc.tensor.matmul(pd[:, 512:1024], lhsT, ct[:, 512:1024], start=True, stop=True)
                    if cls == "a":
                        # DVE min-reduces the whole tile straight from PSUM
                        nc.vector._custom_dve(
                            MIN1,
                            out=mscr[:],
                            in0=pd[:],
                            s0=3.0e38,
                            accum_out=mbuf[:, t : t + 1],
                        )
                    elif cls == "b":
                        # ACT converts right half; DVE reduces PSUM + SBUF
                        cvtb = cvt_pool.tile([128, 512], mybir.dt.bfloat16, tag="cvtb")
                        nc.scalar.copy(cvtb[:], pd[:, 512:1024])
                        nc.vector._custom_dve(
                            MIN2,
                            out=mscr2[:],
                            in0=pd[:, 0:512],
                            in1=cvtb[:],
                            s0=3.0e38,
                            accum_out=mbuf[:, t : t + 1],
                        )
                    else:
                        # ACT converts whole tile; GPSIMD folds 1024 -> 256;
                        # DVE finishes with the fused 2-stream min-reduce
                        cvt = cvt_pool.tile([128, K], mybir.dt.bfloat16, tag="cvtc")
                        nc.scalar.copy(cvt[:], pd[:])
                        gt1 = gp_pool.tile([128, 512], mybir.dt.bfloat16, tag="g1")
                        nc.gpsimd.tensor_tensor(
                            gt1[:], cvt[:, 0:512], cvt[:, 512:1024], mybir.AluOpType.min
                        )
                        gt2 = gp_pool.tile([128, 256], mybir.dt.bfloat16, tag="g2")
                        nc.gpsimd.tensor_tensor(
                            gt2[:], gt1[:, 0:256], gt1[:, 256:512], mybir.AluOpType.min
                        )
                        nc.vector._custom_dve(
                            MIN2,
                            out=mscr3[:],
                            in0=gt2[:, 0:128],
                            in1=gt2[:, 128:256],
                            s0=3.0e38,
                            accum_out=mbuf[:, t : t + 1],
                        )

        nc.sync.dma_start(m_out[:], mbuf[:])
        nc.sync.dma_start(z2_out[:], z2buf[:])

    nc.compile()
    return nc


_NC_CACHE = {}


def kernel(lat, centroids, leak_factor, _want_trace=False):
    lat = np.ascontiguousarray(np.asarray(lat, dtype=np.float32))
    centroids = np.ascontiguousarray(np.asarray(centroids, dtype=np.float32))
    lf = float(np.clip(np.float32(np.asarray(leak_factor)), 1e-3, 1e3))

    if lf not in _NC_CACHE:
        _NC_CACHE[lf] = _build(lf)
    nc = _NC_CACHE[lf]

    # moving operand: rows 0..31 = -2*c^T, rows 32/33 = ||c||^2 split hi/lo
    c2 = (centroids.astype(np.float64) ** 2).sum(axis=1)
    c2_hi = c2.astype(np.float32)
    c2_lo = (c2 - c2_hi).astype(np.float32)
    ct = np.concatenate(
        [(-2.0 * centroids.T).astype(np.float32), c2_hi[None, :], c2_lo[None, :]], axis=0
    )  # [34, 1024]
    ident = np.eye(128, dtype=np.float32)
    ones = np.ones((2, 512), dtype=np.float32)

    in_maps = [
        {
            "lat_in": lat[c * RPC : (c + 1) * RPC],
            "ct_in": ct,
            "id_in": ident,
            "ones_in": ones,
        }
        for c in range(NCORES)
    ]
    res = run_bass_kernel_spmd(
        nc, in_maps, core_ids=list(range(NCORES)), trace=_want_trace
    )

    out = np.concatenate([r["out_lat"] for r in res.results], axis=0)
    msum = np.float64(0.0)
    z2sum = np.float64(0.0)
    for r in res.results:
        msum += r["m_out"].astype(np.float64).sum()
        z2sum += r["z2_out"].astype(np.float64).sum()
    loss = np.float32((1.0 + BETA) * (z2sum + msum) / (B * L))
    kernel.last_results = res
    return out, loss
